# revision 4
# baseline (speedup 1.0000x reference)
"""Trainium2 kernel for nn_EnhancedHybridModel (hybrid MLP + 8-qubit circuit).

Reformulation (exact, up to f32 rounding):
  * BatchNorms are inference-mode -> folded into the adjacent Linear.
  * The quantum circuit after RY-encoding uses shared weights, so it is one
    fixed complex matrix M (256x256).  The encoded state is a REAL product
    vector s[b] = kron_i [cos(pre_i/2), -sin(pre_i/2)].
  * q_out = |M s|^2 @ Z  ->  y = [Re M; Im M] @ s  (512x256 matmul),
    then q_out @ W4eff.T folds with the Z-projection into M4 (512x32):
    h4 = relu(y^2 @ M4 + b4eff).

Data parallel over 8 NeuronCores: batch 65536 -> 8192 rows/core.

v2 optimizations over the first working kernel:
  * cos/-sin via ONE Sin activation with per-partition scale (+-1/2) and
    bias (pi/2 | 0) instead of 6 DVE polynomial ops.
  * final Linear (W6) + b6 accumulated directly in a persistent PSUM bank
    via per-tile padded weights (out row = tile index); single copy at end.
  * small matmuls placed on disjoint PE sub-arrays (tile_position packing):
    A(16x128)@row0, C(64x16)@rows64-127/col0, J(32x16)@rows32-63/col32,
    K(16x16)@rows32-47/col0, B(128x64)@col64 ahead of I(128x32)@col32.
  * squares on the ACT engine (single-read PSUM Square), relus split S/V.
"""

import numpy as np

import concourse.bass as bass
import concourse.mybir as mybir
import concourse.tile as tile
from concourse import bacc
from concourse.masks import make_identity
from concourse.bass_utils import run_bass_kernel_spmd

F32 = mybir.dt.float32
F16 = mybir.dt.float16
AF = mybir.ActivationFunctionType
ALU = mybir.AluOpType

N_CORES = 8
BATCH = 65536
B_CORE = BATCH // N_CORES  # 8192
COLS = 512  # batch columns per tile (one PSUM bank of f32)
NTILES = B_CORE // COLS  # 16

N_QUBITS = 8
N_LAYERS = 3
DIM = 256
EPS = 1e-5

# ---------------------------------------------------------------- host math

_idx = np.arange(DIM)
_CNOT_PERMS = []
for _i in range(N_QUBITS):
    for _j in range(_i + 1, N_QUBITS):
        _c = (_idx >> (N_QUBITS - 1 - _i)) & 1
        _CNOT_PERMS.append(np.where(_c == 1, _idx ^ (1 << (N_QUBITS - 1 - _j)), _idx))
_Z_SIGNS = np.stack(
    [1.0 - 2.0 * ((_idx >> (N_QUBITS - 1 - i)) & 1) for i in range(N_QUBITS)], axis=1
).astype(np.float64)


def _rx(t):
    c, s = np.cos(t / 2), -1j * np.sin(t / 2)
    return np.array([[c, s], [s, c]], np.complex128)


def _ry(t):
    c, s = np.cos(t / 2), np.sin(t / 2)
    return np.array([[c, -s], [s, c]], np.complex128)


def _rz(t):
    e = np.exp(-0.5j * t)
    return np.array([[e, 0], [0, np.conj(e)]], np.complex128)


def _apply_gate(M, G, w):
    # reference einsum('st,bpsq->bptq', U, state): state'[t] = sum_s U[s,t] state[s]
    left = 2**w
    Mr = M.reshape(left, 2, -1, DIM)
    return np.einsum("st,psqj->ptqj", G, Mr).reshape(DIM, DIM)


def _build_circuit_matrix(q_weights):
    qw = np.asarray(q_weights, np.float64)
    M = np.eye(DIM, dtype=np.complex128)
    for l in range(N_LAYERS):
        for i in range(N_QUBITS):
            M = _apply_gate(M, _rx(qw[l, i, 0]), i)
            M = _apply_gate(M, _ry(qw[l, i, 1]), i)
            M = _apply_gate(M, _rz(qw[l, i, 2]), i)
        for perm in _CNOT_PERMS:
            M = M[perm, :]
    return M


def _fold_bn(W, b, g, bt, m, v):
    sc = np.asarray(g, np.float64) / np.sqrt(np.asarray(v, np.float64) + EPS)
    Weff = sc[:, None] * np.asarray(W, np.float64)
    beff = (np.asarray(b, np.float64) - np.asarray(m, np.float64)) * sc + np.asarray(
        bt, np.float64
    )
    return Weff, beff


# WPACK fp16 column layout
_CT0 = 0          # ct: [128, 1024]  (CT[0:128] | CT[128:256])
_CW2 = 1024       # w2: [128, 64]    rows 0-127
_CW1 = 1088       # w1: [16, 128]    rows 0-15
_CW3 = 1216       # w3dup: [64, 16]  rows 64-127
_CW5 = 1232       # w5: [32, 16]     rows 32-63
_CW6 = 1248       # w6pad: [16, 256] rows 32-47, 16 blocks of [16,16]
_CB6 = 1504       # b6row: [1, 16]   row 0
_CM4 = 1520       # m4: 4 chunks [128, 32]
_CEND = 1648


def _prep_consts(inputs):
    f = {k: np.asarray(v, np.float64) for k, v in inputs.items() if k != "x"}
    W1e, b1e = _fold_bn(f["W1"], f["b1"], f["g1"], f["bt1"], f["m1"], f["v1"])
    W2e, b2e = _fold_bn(f["W2"], f["b2"], f["g2"], f["bt2"], f["m2"], f["v2"])
    W4e, b4e = _fold_bn(f["W4"], f["b4"], f["g4"], f["bt4"], f["m4"], f["v4"])
    M = _build_circuit_matrix(f["q_weights"])
    C = np.concatenate([M.real, M.imag], axis=0)  # (512, 256)
    Zst = np.concatenate([_Z_SIGNS, _Z_SIGNS], axis=0)  # (512, 8)
    M4 = Zst @ W4e.T  # (512, 32)

    bf = np.float16
    wpk = np.zeros((128, _CEND), bf)
    CT = np.ascontiguousarray(C.T).astype(bf)  # (256, 512)
    wpk[:, _CT0:_CT0 + 512] = CT[0:128]
    wpk[:, _CT0 + 512:_CT0 + 1024] = CT[128:256]
    wpk[0:128, _CW2:_CW2 + 64] = np.ascontiguousarray(W2e.T).astype(bf)
    wpk[0:16, _CW1:_CW1 + 128] = np.ascontiguousarray(W1e.T).astype(bf)
    w3t = np.concatenate([f["W3"].T, f["W3"].T], axis=1)  # (64, 16)
    wpk[64:128, _CW3:_CW3 + 16] = np.ascontiguousarray(w3t).astype(bf)
    wpk[32:64, _CW5:_CW5 + 16] = np.ascontiguousarray(f["W5"].T).astype(bf)
    w6 = np.asarray(f["W6"], np.float64).reshape(16)
    for i in range(NTILES):
        wpk[32:48, _CW6 + 16 * i + i] = w6.astype(bf)  # col i of block i
    wpk[0:1, _CB6:_CB6 + 16] = np.float64(f["b6"].reshape(1)[0]).astype(bf)
    M4b = M4.astype(bf)  # (512, 32)
    for c in range(4):
        wpk[:, _CM4 + 32 * c:_CM4 + 32 * (c + 1)] = M4b[128 * c:128 * (c + 1)]

    # BIASES f32 [128, 16]:
    # col0 b1 (rows0-127) | col1 b2 (rows64-127) | col2 b3dup (rows0-15)
    # col3 b4 (rows32-63) | col4 b5 (rows32-47)
    # col6 sin-scale (rows0-15: 8x +.5, 8x -.5) | col7 sin-bias (pi/2 | 0)
    bs = np.zeros((128, 16), np.float32)
    bs[0:128, 0] = b1e
    bs[64:128, 1] = b2e
    bs[0:16, 2] = np.concatenate([f["b3"], f["b3"]])
    bs[32:64, 3] = b4e
    bs[32:48, 4] = f["b5"]
    bs[0:8, 6] = 0.5
    bs[8:16, 6] = -0.5
    bs[0:8, 7] = np.pi / 2
    bs[8:16, 7] = 0.0
    return {"WPACK": wpk, "BIASES": bs}


# ------------------------------------------------------------- bass program


def _ap(t, offset, dims):
    """Custom free-dim access pattern on a tile: keep its partition dim."""
    a = t[:]
    return bass.AP(a.tensor, a.offset + offset, [list(a.ap[0])] + [list(d) for d in dims])


def _build_nc():
    nc = bacc.Bacc("TRN2", target_bir_lowering=False, debug=False)

    xt = nc.dram_tensor("xt", [16, B_CORE], F16, kind="ExternalInput")
    wpk_d = nc.dram_tensor("WPACK", [128, _CEND], F16, kind="ExternalInput")
    bs_d = nc.dram_tensor("BIASES", [128, 16], F32, kind="ExternalInput")
    out_d = nc.dram_tensor("out", [NTILES, COLS], F32, kind="ExternalOutput")

    with tile.TileContext(nc) as tc:
        with (
            tc.tile_pool(name="const", bufs=1) as cp,
            tc.tile_pool(name="work", bufs=6) as wp,
            tc.tile_pool(name="pmlp", bufs=2, space="PSUM") as pmlp,
            tc.tile_pool(name="pmlb", bufs=2, space="PSUM") as pmlb,
            tc.tile_pool(name="py", bufs=2, space="PSUM") as py,
            tc.tile_pool(name="ptr", bufs=1, space="PSUM") as ptr,
            tc.tile_pool(name="pacc", bufs=1, space="PSUM") as pacc,
        ):
            ident = cp.tile([128, 128], F16)
            make_identity(nc, ident[:])
            wpk = cp.tile([128, _CEND], F16)
            nc.scalar.dma_start(wpk[:], wpk_d[:])
            bs = cp.tile([128, 16], F32)
            nc.sync.dma_start(bs[:], bs_d[:])
            ones = cp.tile([1, COLS], F16)
            nc.vector.memset(ones[:], 1.0)

            w2 = wpk[:, _CW2:_CW2 + 64]
            w1 = wpk[0:16, _CW1:_CW1 + 128]
            w3 = wpk[64:128, _CW3:_CW3 + 16]
            w5 = wpk[32:64, _CW5:_CW5 + 16]
            b6row = wpk[0:1, _CB6:_CB6 + 16]
            ct = wpk[:, _CT0:_CT0 + 1024]
            m4 = wpk[:, _CM4:_CM4 + 128]

            b1 = bs[0:128, 0:1]
            b2 = bs[64:128, 1:2]
            b3 = bs[0:16, 2:3]
            b4 = bs[32:64, 3:4]
            b5 = bs[32:48, 4:5]
            sinsc = bs[0:16, 6:7]
            sinb = bs[0:16, 7:8]

            xg = []
            for g in range(4):
                xg.append(cp.tile([16, 4 * COLS], F16, name=f"xg{g}", tag=f"xg{g}"))
                nc.sync.dma_start(xg[g][:], xt[:, 4 * COLS * g:4 * COLS * (g + 1)])

            mm = nc.tensor.matmul

            # persistent output accumulator: row i = tile i's output;
            # seeded with b6 via a rank-1 matmul so accumulation adds it once.
            out_acc = pacc.tile([16, COLS], F32)
            mm(out_acc[:], b6row, ones[:], start=True, stop=False,
               skip_group_check=True)

            h1 = [None] * NTILES
            h2 = [None] * NTILES
            preg = [None] * NTILES
            csA = [None] * NTILES
            cs = [None] * NTILES
            sB = [None] * NTILES
            sT0 = [None] * NTILES
            sT1 = [None] * NTILES
            sqa = [None] * NTILES
            sqb = [None] * NTILES
            h4 = [None] * NTILES
            h5 = [None] * NTILES

            LAG = dict(A=2, B=3, C=4, D=5, E=6, F=7, G1=8, G2=9, H=10,
                       I=11, J=12, K=13)

            def live(ph, t):
                i = t - LAG[ph]
                return i if 0 <= i < NTILES else None

            for t in range(NTILES + 14):
                # A: h1 = relu(W1eff x + b1); PE row-strip 0, all cols.
                i = live("A", t)
                if i is not None:
                    h1p = pmlp.tile([128, COLS], F32, tag="mlp")
                    mm(h1p[:], w1, xg[i // 4][:, COLS * (i % 4):COLS * (i % 4 + 1)])
                    h1[i] = wp.tile([128, COLS], F16, tag="h1", name="h1", bufs=2)
                    nc.vector.tensor_scalar(h1[i][:], h1p[:], b1, 0.0, ALU.add, ALU.max)

                # B: h2 = relu(W2eff h1 + b2) at partitions 64-127 (col 64).
                i = live("B", t)
                if i is not None:
                    h2p = pmlp.tile([128, COLS], F32, tag="mlp")
                    mm(h2p[64:128, :], w2, h1[i][:])
                    h2[i] = wp.tile([128, COLS], F16, tag="h2", name="h2", bufs=2)
                    nc.vector.tensor_scalar(h2[i][64:128, :], h2p[64:128, :], b2,
                                            0.0, ALU.add, ALU.max)

                # C: pre = tanh(W3 h2 + b3) (16 rows = angles duplicated);
                # PE sub-array rows 64-127 x col 0.
                i = live("C", t)
                if i is not None:
                    prp = pmlp.tile([128, COLS], F32, tag="mlp")
                    mm(prp[0:16, :], w3, h2[i][64:128, :])
                    preg[i] = wp.tile([16, COLS], F16, tag="pre", name="pre", bufs=2)
                    nc.scalar.activation(preg[i][:], prp[0:16, :], AF.Tanh, bias=b3)

                # D: csA = [cos(t/2); -sin(t/2)] via one Sin activation with
                # per-partition scale (+-1/2) and bias (pi/2 | 0).
                i = live("D", t)
                if i is not None:
                    csA[i] = wp.tile([16, COLS], F16, tag="csA", name="csA", bufs=2)
                    nc.scalar.activation(csA[i][:], preg[i][:], AF.Sin,
                                         bias=sinb, scale=sinsc)

                # E: transpose csA to batch-major cs [128, 64].
                i = live("E", t)
                if i is not None:
                    cs_ps = ptr.tile([128, 64], F16, tag="tr")
                    for b in range(4):
                        nc.tensor.transpose(
                            cs_ps[:, 16 * b:16 * (b + 1)],
                            csA[i][:, 128 * b:128 * (b + 1)],
                            ident[0:16, 0:16],
                        )
                    cs[i] = wp.tile([128, 64], F16, tag="cs", name="cs", bufs=2)
                    nc.vector.tensor_copy(cs[i][:], cs_ps[:])

                # F: product-state build (batch-major), gpsimd + vector.
                i = live("F", t)
                if i is not None:
                    qp = wp.tile([128, 64], F16, tag="qp", name="qp", bufs=2)
                    for a in range(2):
                        nc.gpsimd.tensor_mul(
                            _ap(qp, 2 * a, [[16, 4], [4, 4], [1, 2]]),
                            _ap(cs[i], 8 * a, [[16, 4], [2, 4], [0, 2]]),
                            _ap(cs[i], 1, [[16, 4], [2, 4], [8, 2]]),
                        )
                    uv = wp.tile([128, 128], F16, tag="uv", name="uv", bufs=2)
                    nc.gpsimd.tensor_mul(
                        _ap(uv, 0, [[16, 8], [4, 4], [1, 4]]),
                        _ap(qp, 0, [[8, 8], [1, 4], [0, 4]]),
                        _ap(qp, 4, [[8, 8], [0, 4], [1, 4]]),
                    )
                    sB[i] = wp.tile([128, 1024], F16, tag="sB", name="sB", bufs=3)
                    nc.vector.tensor_mul(
                        _ap(sB[i], 0, [[256, 2], [16, 16], [1, 16]]),
                        _ap(uv, 0, [[32, 2], [1, 16], [0, 16]]),
                        _ap(uv, 16, [[32, 2], [0, 16], [1, 16]]),
                    )
                    nc.gpsimd.tensor_mul(
                        _ap(sB[i], 512, [[256, 2], [16, 16], [1, 16]]),
                        _ap(uv, 64, [[32, 2], [1, 16], [0, 16]]),
                        _ap(uv, 80, [[32, 2], [0, 16], [1, 16]]),
                    )

                # G1/G2: transpose sB to state-major sT0 (rows 0-127) and
                # sT1 (rows 128-255), one quad + copy per phase.
                i = live("G1", t)
                if i is not None:
                    ps0 = ptr.tile([128, COLS], F16, tag="tr")
                    for b in range(4):
                        nc.tensor.transpose(ps0[:, 128 * b:128 * (b + 1)],
                                            sB[i][:, 256 * b:256 * b + 128], ident[:])
                    sT0[i] = wp.tile([128, COLS], F16, tag="sT0", name="sT0", bufs=3)
                    nc.vector.tensor_copy(sT0[i][:], ps0[:])

                i = live("G2", t)
                if i is not None:
                    ps1 = ptr.tile([128, COLS], F16, tag="tr")
                    for b in range(4):
                        nc.tensor.transpose(ps1[:, 128 * b:128 * (b + 1)],
                                            sB[i][:, 256 * b + 128:256 * (b + 1)], ident[:])
                    sT1[i] = wp.tile([128, COLS], F16, tag="sT1", name="sT1", bufs=3)
                    nc.vector.tensor_copy(sT1[i][:], ps1[:])

                # H: y = C s (8 matmuls), squares on ACT engine.
                i = live("H", t)
                if i is not None:
                    sqa[i] = wp.tile([128, 1024], F16, tag="sqa", name="sqa", bufs=2)
                    sqb[i] = wp.tile([128, 1024], F16, tag="sqb", name="sqb", bufs=2)
                    for mc in range(4):
                        yp = py.tile([128, COLS], F32, tag="y")
                        mm(yp[:], ct[:, 128 * mc:128 * (mc + 1)], sT0[i][:],
                           start=True, stop=False)
                        mm(yp[:], ct[:, 512 + 128 * mc:512 + 128 * (mc + 1)], sT1[i][:],
                           start=False, stop=True)
                        dst = (sqa if mc < 2 else sqb)[i][:, 512 * (mc % 2):512 * (mc % 2 + 1)]
                        nc.scalar.activation(dst, yp[:], AF.Square)

                # I: h4 = relu(y^2 @ M4 + b4) at partitions 32-63 (col 32).
                i = live("I", t)
                if i is not None:
                    h4p = pmlb.tile([128, COLS], F32, tag="mlb")
                    for mc in range(4):
                        srct = (sqa if mc < 2 else sqb)[i][:, 512 * (mc % 2):512 * (mc % 2 + 1)]
                        mm(h4p[32:64, :], m4[:, 32 * mc:32 * (mc + 1)], srct,
                           start=(mc == 0), stop=(mc == 3))
                    h4[i] = wp.tile([64, COLS], F16, tag="h4", name="h4", bufs=2)
                    nc.scalar.activation(h4[i][32:64, :], h4p[32:64, :], AF.Relu,
                                         bias=b4)

                # J: h5 = relu(W5 h4 + b5); PE rows 32-63 x col 32.
                i = live("J", t)
                if i is not None:
                    h5p = pmlb.tile([128, COLS], F32, tag="mlb")
                    mm(h5p[32:48, :], w5, h4[i][32:64, :])
                    h5[i] = wp.tile([48, COLS], F16, tag="h5", name="h5", bufs=2)
                    nc.vector.tensor_scalar(h5[i][32:48, :], h5p[32:48, :], b5,
                                            0.0, ALU.add, ALU.max)

                # K: out row i += W6 h5 (padded weights select row i);
                # PE rows 32-47 x col 0.
                i = live("K", t)
                if i is not None:
                    w6i = wpk[32:48, _CW6 + 16 * i:_CW6 + 16 * (i + 1)]
                    mm(out_acc[:], w6i, h5[i][32:48, :], start=False,
                       stop=(i == NTILES - 1), skip_group_check=True)

            out_sb = cp.tile([NTILES, COLS], F32)
            nc.vector.tensor_copy(out_sb[:], out_acc[:])
            nc.sync.dma_start(out_d[:], out_sb[:])

    nc.compile()
    return nc


_NC_CACHE = []

# test-harness hooks (unused in grading): set _TRACE to profile; the full
# BassKernelResults of the last run lands in _LAST_RESULTS[0].
_TRACE = False
_LAST_RESULTS = []


def _get_nc():
    if not _NC_CACHE:
        _NC_CACHE.append(_build_nc())
    return _NC_CACHE[0]


def kernel(**inputs):
    consts = _prep_consts(inputs)
    x = np.asarray(inputs["x"], np.float32)  # (65536, 16)
    xt_full = np.ascontiguousarray(x.T.astype(np.float16))  # (16, 65536)

    nc = _get_nc()
    in_maps = []
    for c in range(N_CORES):
        m = {"xt": np.ascontiguousarray(xt_full[:, c * B_CORE:(c + 1) * B_CORE])}
        m.update(consts)
        in_maps.append(m)
    res = run_bass_kernel_spmd(nc, in_maps, list(range(N_CORES)), trace=_TRACE)
    _LAST_RESULTS.clear()
    _LAST_RESULTS.append(res)
    out = np.concatenate([r["out"].reshape(B_CORE) for r in res.results])
    return out.reshape(BATCH, 1).astype(np.float32)


# revision 5
# speedup vs baseline: 1.1829x; 1.1829x over previous
"""Trainium2 kernel for nn_EnhancedHybridModel (hybrid MLP + 8-qubit circuit).

Reformulation (exact, up to f32 rounding):
  * BatchNorms are inference-mode -> folded into the adjacent Linear.
  * The quantum circuit after RY-encoding uses shared weights, so it is one
    fixed complex matrix M (256x256).  The encoded state is a REAL product
    vector s[b] = kron_i [cos(pre_i/2), -sin(pre_i/2)].
  * q_out = |M s|^2 @ Z  ->  y = [Re M; Im M] @ s  (512x256 matmul),
    then q_out @ W4eff.T folds with the Z-projection into M4 (512x32):
    h4 = relu(y^2 @ M4 + b4eff).

Data parallel over 8 NeuronCores: batch 65536 -> 8192 rows/core.

v2 optimizations over the first working kernel:
  * cos/-sin via ONE Sin activation with per-partition scale (+-1/2) and
    bias (pi/2 | 0) instead of 6 DVE polynomial ops.
  * final Linear (W6) + b6 accumulated directly in a persistent PSUM bank
    via per-tile padded weights (out row = tile index); single copy at end.
  * small matmuls placed on disjoint PE sub-arrays (tile_position packing):
    A(16x128)@row0, C(64x16)@rows64-127/col0, J(32x16)@rows32-63/col32,
    K(16x16)@rows32-47/col0, B(128x64)@col64 ahead of I(128x32)@col32.
  * squares on the ACT engine (single-read PSUM Square), relus split S/V.
"""

import numpy as np

import concourse.bass as bass
import concourse.mybir as mybir
import concourse.tile as tile
from concourse import bacc
from concourse.masks import make_identity
from concourse.bass_utils import run_bass_kernel_spmd

F32 = mybir.dt.float32
F16 = mybir.dt.float16
AF = mybir.ActivationFunctionType
ALU = mybir.AluOpType

N_CORES = 8
BATCH = 65536
B_CORE = BATCH // N_CORES  # 8192
COLS = 512  # batch columns per tile (one PSUM bank of f32)
NTILES = B_CORE // COLS  # 16

N_QUBITS = 8
N_LAYERS = 3
DIM = 256
EPS = 1e-5

# ---------------------------------------------------------------- host math

_idx = np.arange(DIM)
_CNOT_PERMS = []
for _i in range(N_QUBITS):
    for _j in range(_i + 1, N_QUBITS):
        _c = (_idx >> (N_QUBITS - 1 - _i)) & 1
        _CNOT_PERMS.append(np.where(_c == 1, _idx ^ (1 << (N_QUBITS - 1 - _j)), _idx))
_Z_SIGNS = np.stack(
    [1.0 - 2.0 * ((_idx >> (N_QUBITS - 1 - i)) & 1) for i in range(N_QUBITS)], axis=1
).astype(np.float64)


def _rx(t):
    c, s = np.cos(t / 2), -1j * np.sin(t / 2)
    return np.array([[c, s], [s, c]], np.complex128)


def _ry(t):
    c, s = np.cos(t / 2), np.sin(t / 2)
    return np.array([[c, -s], [s, c]], np.complex128)


def _rz(t):
    e = np.exp(-0.5j * t)
    return np.array([[e, 0], [0, np.conj(e)]], np.complex128)


def _apply_gate(M, G, w):
    # reference einsum('st,bpsq->bptq', U, state): state'[t] = sum_s U[s,t] state[s]
    left = 2**w
    Mr = M.reshape(left, 2, -1, DIM)
    return np.einsum("st,psqj->ptqj", G, Mr).reshape(DIM, DIM)


def _build_circuit_matrix(q_weights):
    qw = np.asarray(q_weights, np.float64)
    M = np.eye(DIM, dtype=np.complex128)
    for l in range(N_LAYERS):
        for i in range(N_QUBITS):
            M = _apply_gate(M, _rx(qw[l, i, 0]), i)
            M = _apply_gate(M, _ry(qw[l, i, 1]), i)
            M = _apply_gate(M, _rz(qw[l, i, 2]), i)
        for perm in _CNOT_PERMS:
            M = M[perm, :]
    return M


def _fold_bn(W, b, g, bt, m, v):
    sc = np.asarray(g, np.float64) / np.sqrt(np.asarray(v, np.float64) + EPS)
    Weff = sc[:, None] * np.asarray(W, np.float64)
    beff = (np.asarray(b, np.float64) - np.asarray(m, np.float64)) * sc + np.asarray(
        bt, np.float64
    )
    return Weff, beff


# WPACK fp16 column layout
_CT0 = 0          # ct: [128, 1024]  (CT[0:128] | CT[128:256])
_CW2 = 1024       # w2: [128, 64]    rows 0-127
_CW1 = 1088       # w1: [16, 128]    rows 0-15
_CW3 = 1216       # w3dup: [64, 16]  rows 64-127
_CW5 = 1232       # w5: [32, 16]     rows 32-63
_CW6 = 1248       # w6pad: [16, 256] rows 32-47, 16 blocks of [16,16]
_CB6 = 1504       # b6row: [1, 16]   row 0
_CM4 = 1520       # m4: 4 chunks [128, 32]
_CEND = 1648


def _prep_consts(inputs):
    f = {k: np.asarray(v, np.float64) for k, v in inputs.items() if k != "x"}
    W1e, b1e = _fold_bn(f["W1"], f["b1"], f["g1"], f["bt1"], f["m1"], f["v1"])
    W2e, b2e = _fold_bn(f["W2"], f["b2"], f["g2"], f["bt2"], f["m2"], f["v2"])
    W4e, b4e = _fold_bn(f["W4"], f["b4"], f["g4"], f["bt4"], f["m4"], f["v4"])
    M = _build_circuit_matrix(f["q_weights"])
    C = np.concatenate([M.real, M.imag], axis=0)  # (512, 256)
    Zst = np.concatenate([_Z_SIGNS, _Z_SIGNS], axis=0)  # (512, 8)
    M4 = Zst @ W4e.T  # (512, 32)

    bf = np.float16
    wpk = np.zeros((128, _CEND), bf)
    CT = np.ascontiguousarray(C.T).astype(bf)  # (256, 512)
    wpk[:, _CT0:_CT0 + 512] = CT[0:128]
    wpk[:, _CT0 + 512:_CT0 + 1024] = CT[128:256]
    wpk[0:128, _CW2:_CW2 + 64] = np.ascontiguousarray(W2e.T).astype(bf)
    wpk[0:16, _CW1:_CW1 + 128] = np.ascontiguousarray(W1e.T).astype(bf)
    w3t = np.concatenate([f["W3"].T, f["W3"].T], axis=1)  # (64, 16)
    wpk[64:128, _CW3:_CW3 + 16] = np.ascontiguousarray(w3t).astype(bf)
    wpk[32:64, _CW5:_CW5 + 16] = np.ascontiguousarray(f["W5"].T).astype(bf)
    w6 = np.asarray(f["W6"], np.float64).reshape(16)
    for i in range(NTILES):
        wpk[32:48, _CW6 + 16 * i + i] = w6.astype(bf)  # col i of block i
    wpk[0:1, _CB6:_CB6 + 16] = np.float64(f["b6"].reshape(1)[0]).astype(bf)
    M4b = M4.astype(bf)  # (512, 32)
    for c in range(4):
        wpk[:, _CM4 + 32 * c:_CM4 + 32 * (c + 1)] = M4b[128 * c:128 * (c + 1)]

    # BIASES f32 [128, 16]:
    # col0 b1 (rows0-127) | col1 b2 (rows64-127) | col2 b3dup (rows0-15)
    # col3 b4 (rows32-63) | col4 b5 (rows32-47)
    # col6 sin-scale (rows0-15: 8x +.5, 8x -.5) | col7 sin-bias (pi/2 | 0)
    bs = np.zeros((128, 16), np.float32)
    bs[0:128, 0] = b1e
    bs[64:128, 1] = b2e
    bs[0:16, 2] = np.concatenate([f["b3"], f["b3"]])
    bs[32:64, 3] = b4e
    bs[32:48, 4] = f["b5"]
    bs[0:8, 6] = 0.5
    bs[8:16, 6] = -0.5
    bs[0:8, 7] = np.pi / 2
    bs[8:16, 7] = 0.0
    return {"WPACK": wpk, "BIASES": bs}


# ------------------------------------------------------------- bass program


def _ap(t, offset, dims):
    """Custom free-dim access pattern on a tile: keep its partition dim."""
    a = t[:]
    return bass.AP(a.tensor, a.offset + offset, [list(a.ap[0])] + [list(d) for d in dims])


class _Bacc(bacc.Bacc):
    """Bacc that pins all activations to one table set.

    The greedy act-table pass picks the first set containing each function;
    Tanh lands in set 0 and Sin in set 9, thrashing ACT_TABLE_LOAD (~1.3us)
    twice per pipeline iteration.  'silu_and_others' contains sin, tanh,
    square AND relu; emptying every other set (names/indices kept, so the
    emitted act_func_set_id still matches act_info.json) forces one load.
    """

    def insert_act_table_loads(self):
        import bass_rust as _bass_rust
        from concourse.hw_specs import get_activation_tables

        has_activation = any(
            isinstance(i, mybir.InstActivation)
            for b in self.main_func.blocks
            for i in b.instructions
        )
        if not has_activation:
            return
        tables = [
            (n, (s if n == "silu_and_others" else set()))
            for n, s in get_activation_tables(self.m.arch).items()
        ]
        _bass_rust.insert_act_table_loads(self, tables)


def _build_nc():
    nc = _Bacc("TRN2", target_bir_lowering=False, debug=False)

    xt = nc.dram_tensor("xt", [16, B_CORE], F16, kind="ExternalInput")
    wpk_d = nc.dram_tensor("WPACK", [128, _CEND], F16, kind="ExternalInput")
    bs_d = nc.dram_tensor("BIASES", [128, 16], F32, kind="ExternalInput")
    out_d = nc.dram_tensor("out", [NTILES, COLS], F32, kind="ExternalOutput")

    with tile.TileContext(nc) as tc:
        with (
            tc.tile_pool(name="const", bufs=1) as cp,
            tc.tile_pool(name="work", bufs=6) as wp,
            tc.tile_pool(name="pmlp", bufs=2, space="PSUM") as pmlp,
            tc.tile_pool(name="pmlb", bufs=2, space="PSUM") as pmlb,
            tc.tile_pool(name="py", bufs=2, space="PSUM") as py,
            tc.tile_pool(name="ptr", bufs=1, space="PSUM") as ptr,
            tc.tile_pool(name="pacc", bufs=1, space="PSUM") as pacc,
        ):
            ident = cp.tile([128, 128], F16)
            make_identity(nc, ident[:])
            wpk = cp.tile([128, _CEND], F16)
            nc.scalar.dma_start(wpk[:], wpk_d[:])
            bs = cp.tile([128, 16], F32)
            nc.sync.dma_start(bs[:], bs_d[:])
            ones = cp.tile([1, COLS], F16)
            nc.vector.memset(ones[:], 1.0)

            w2 = wpk[:, _CW2:_CW2 + 64]
            w1 = wpk[0:16, _CW1:_CW1 + 128]
            w3 = wpk[64:128, _CW3:_CW3 + 16]
            w5 = wpk[32:64, _CW5:_CW5 + 16]
            b6row = wpk[0:1, _CB6:_CB6 + 16]
            ct = wpk[:, _CT0:_CT0 + 1024]
            m4 = wpk[:, _CM4:_CM4 + 128]

            b1 = bs[0:128, 0:1]
            b2 = bs[64:128, 1:2]
            b3 = bs[0:16, 2:3]
            b4 = bs[32:64, 3:4]
            b5 = bs[32:48, 4:5]
            sinsc = bs[0:16, 6:7]
            sinb = bs[0:16, 7:8]

            xg = []
            for g in range(4):
                xg.append(cp.tile([16, 4 * COLS], F16, name=f"xg{g}", tag=f"xg{g}"))
                nc.sync.dma_start(xg[g][:], xt[:, 4 * COLS * g:4 * COLS * (g + 1)])

            mm = nc.tensor.matmul

            # persistent output accumulator: row i = tile i's output;
            # seeded with b6 via a rank-1 matmul so accumulation adds it once.
            out_acc = pacc.tile([16, COLS], F32)
            mm(out_acc[:], b6row, ones[:], start=True, stop=False,
               skip_group_check=True)

            h1 = [None] * NTILES
            h2 = [None] * NTILES
            preg = [None] * NTILES
            csA = [None] * NTILES
            cs = [None] * NTILES
            sB = [None] * NTILES
            sT0 = [None] * NTILES
            sT1 = [None] * NTILES
            sqa = [None] * NTILES
            sqb = [None] * NTILES
            h4 = [None] * NTILES
            h5 = [None] * NTILES

            LAG = dict(A=2, B=3, C=4, D=5, E=6, F=7, G1=8, G2=9, H=10,
                       I=11, J=12, K=13)

            def live(ph, t):
                i = t - LAG[ph]
                return i if 0 <= i < NTILES else None

            for t in range(NTILES + 14):
                # A: h1 = relu(W1eff x + b1); PE row-strip 0, all cols.
                i = live("A", t)
                if i is not None:
                    h1p = pmlp.tile([128, COLS], F32, tag="mlp")
                    mm(h1p[:], w1, xg[i // 4][:, COLS * (i % 4):COLS * (i % 4 + 1)])
                    h1[i] = wp.tile([128, COLS], F16, tag="h1", name="h1", bufs=2)
                    nc.vector.tensor_scalar(h1[i][:], h1p[:], b1, 0.0, ALU.add, ALU.max)

                # B: h2 = relu(W2eff h1 + b2) at partitions 64-127 (col 64).
                i = live("B", t)
                if i is not None:
                    h2p = pmlp.tile([128, COLS], F32, tag="mlp")
                    mm(h2p[64:128, :], w2, h1[i][:])
                    h2[i] = wp.tile([128, COLS], F16, tag="h2", name="h2", bufs=2)
                    nc.vector.tensor_scalar(h2[i][64:128, :], h2p[64:128, :], b2,
                                            0.0, ALU.add, ALU.max)

                # C: pre = tanh(W3 h2 + b3) (16 rows = angles duplicated);
                # PE sub-array rows 64-127 x col 0.
                i = live("C", t)
                if i is not None:
                    prp = pmlp.tile([128, COLS], F32, tag="mlp")
                    mm(prp[0:16, :], w3, h2[i][64:128, :])
                    preg[i] = wp.tile([16, COLS], F16, tag="pre", name="pre", bufs=2)
                    nc.scalar.activation(preg[i][:], prp[0:16, :], AF.Tanh, bias=b3)

                # D: csA = [cos(t/2); -sin(t/2)] via one Sin activation with
                # per-partition scale (+-1/2) and bias (pi/2 | 0).
                i = live("D", t)
                if i is not None:
                    csA[i] = wp.tile([16, COLS], F16, tag="csA", name="csA", bufs=2)
                    nc.scalar.activation(csA[i][:], preg[i][:], AF.Sin,
                                         bias=sinb, scale=sinsc)

                # E: transpose csA to batch-major cs [128, 64].
                i = live("E", t)
                if i is not None:
                    cs_ps = ptr.tile([128, 64], F16, tag="tr")
                    for b in range(4):
                        nc.tensor.transpose(
                            cs_ps[:, 16 * b:16 * (b + 1)],
                            csA[i][:, 128 * b:128 * (b + 1)],
                            ident[0:16, 0:16],
                        )
                    cs[i] = wp.tile([128, 64], F16, tag="cs", name="cs", bufs=2)
                    nc.vector.tensor_copy(cs[i][:], cs_ps[:])

                # F: product-state build (batch-major), gpsimd + vector.
                i = live("F", t)
                if i is not None:
                    qp = wp.tile([128, 64], F16, tag="qp", name="qp", bufs=2)
                    for a in range(2):
                        nc.gpsimd.tensor_mul(
                            _ap(qp, 2 * a, [[16, 4], [4, 4], [1, 2]]),
                            _ap(cs[i], 8 * a, [[16, 4], [2, 4], [0, 2]]),
                            _ap(cs[i], 1, [[16, 4], [2, 4], [8, 2]]),
                        )
                    uv = wp.tile([128, 128], F16, tag="uv", name="uv", bufs=2)
                    nc.gpsimd.tensor_mul(
                        _ap(uv, 0, [[16, 8], [4, 4], [1, 4]]),
                        _ap(qp, 0, [[8, 8], [1, 4], [0, 4]]),
                        _ap(qp, 4, [[8, 8], [0, 4], [1, 4]]),
                    )
                    sB[i] = wp.tile([128, 1024], F16, tag="sB", name="sB", bufs=3)
                    nc.vector.tensor_mul(
                        _ap(sB[i], 0, [[256, 2], [16, 16], [1, 16]]),
                        _ap(uv, 0, [[32, 2], [1, 16], [0, 16]]),
                        _ap(uv, 16, [[32, 2], [0, 16], [1, 16]]),
                    )
                    nc.gpsimd.tensor_mul(
                        _ap(sB[i], 512, [[256, 2], [16, 16], [1, 16]]),
                        _ap(uv, 64, [[32, 2], [1, 16], [0, 16]]),
                        _ap(uv, 80, [[32, 2], [0, 16], [1, 16]]),
                    )

                # G1/G2: transpose sB to state-major sT0 (rows 0-127) and
                # sT1 (rows 128-255), one quad + copy per phase.
                i = live("G1", t)
                if i is not None:
                    ps0 = ptr.tile([128, COLS], F16, tag="tr")
                    for b in range(4):
                        nc.tensor.transpose(ps0[:, 128 * b:128 * (b + 1)],
                                            sB[i][:, 256 * b:256 * b + 128], ident[:])
                    sT0[i] = wp.tile([128, COLS], F16, tag="sT0", name="sT0", bufs=3)
                    nc.vector.tensor_copy(sT0[i][:], ps0[:])

                i = live("G2", t)
                if i is not None:
                    ps1 = ptr.tile([128, COLS], F16, tag="tr")
                    for b in range(4):
                        nc.tensor.transpose(ps1[:, 128 * b:128 * (b + 1)],
                                            sB[i][:, 256 * b + 128:256 * (b + 1)], ident[:])
                    sT1[i] = wp.tile([128, COLS], F16, tag="sT1", name="sT1", bufs=3)
                    nc.vector.tensor_copy(sT1[i][:], ps1[:])

                # H: y = C s (8 matmuls), squares on ACT engine.
                i = live("H", t)
                if i is not None:
                    sqa[i] = wp.tile([128, 1024], F16, tag="sqa", name="sqa", bufs=2)
                    sqb[i] = wp.tile([128, 1024], F16, tag="sqb", name="sqb", bufs=2)
                    for mc in range(4):
                        yp = py.tile([128, COLS], F32, tag="y")
                        mm(yp[:], ct[:, 128 * mc:128 * (mc + 1)], sT0[i][:],
                           start=True, stop=False)
                        mm(yp[:], ct[:, 512 + 128 * mc:512 + 128 * (mc + 1)], sT1[i][:],
                           start=False, stop=True)
                        dst = (sqa if mc < 2 else sqb)[i][:, 512 * (mc % 2):512 * (mc % 2 + 1)]
                        nc.scalar.activation(dst, yp[:], AF.Square)

                # I: h4 = relu(y^2 @ M4 + b4) at partitions 32-63 (col 32).
                i = live("I", t)
                if i is not None:
                    h4p = pmlb.tile([128, COLS], F32, tag="mlb")
                    for mc in range(4):
                        srct = (sqa if mc < 2 else sqb)[i][:, 512 * (mc % 2):512 * (mc % 2 + 1)]
                        mm(h4p[32:64, :], m4[:, 32 * mc:32 * (mc + 1)], srct,
                           start=(mc == 0), stop=(mc == 3))
                    h4[i] = wp.tile([64, COLS], F16, tag="h4", name="h4", bufs=2)
                    nc.scalar.activation(h4[i][32:64, :], h4p[32:64, :], AF.Relu,
                                         bias=b4)

                # J: h5 = relu(W5 h4 + b5); PE rows 32-63 x col 32.
                i = live("J", t)
                if i is not None:
                    h5p = pmlb.tile([128, COLS], F32, tag="mlb")
                    mm(h5p[32:48, :], w5, h4[i][32:64, :])
                    h5[i] = wp.tile([48, COLS], F16, tag="h5", name="h5", bufs=2)
                    nc.vector.tensor_scalar(h5[i][32:48, :], h5p[32:48, :], b5,
                                            0.0, ALU.add, ALU.max)

                # K: out row i += W6 h5 (padded weights select row i);
                # PE rows 32-47 x col 0.
                i = live("K", t)
                if i is not None:
                    w6i = wpk[32:48, _CW6 + 16 * i:_CW6 + 16 * (i + 1)]
                    mm(out_acc[:], w6i, h5[i][32:48, :], start=False,
                       stop=(i == NTILES - 1), skip_group_check=True)

            out_sb = cp.tile([NTILES, COLS], F32)
            nc.vector.tensor_copy(out_sb[:], out_acc[:])
            nc.sync.dma_start(out_d[:], out_sb[:])

    nc.compile()
    return nc


_NC_CACHE = []

# test-harness hooks (unused in grading): set _TRACE to profile; the full
# BassKernelResults of the last run lands in _LAST_RESULTS[0].
_TRACE = False
_LAST_RESULTS = []


def _get_nc():
    if not _NC_CACHE:
        _NC_CACHE.append(_build_nc())
    return _NC_CACHE[0]


def kernel(**inputs):
    consts = _prep_consts(inputs)
    x = np.asarray(inputs["x"], np.float32)  # (65536, 16)
    xt_full = np.ascontiguousarray(x.T.astype(np.float16))  # (16, 65536)

    nc = _get_nc()
    in_maps = []
    for c in range(N_CORES):
        m = {"xt": np.ascontiguousarray(xt_full[:, c * B_CORE:(c + 1) * B_CORE])}
        m.update(consts)
        in_maps.append(m)
    res = run_bass_kernel_spmd(nc, in_maps, list(range(N_CORES)), trace=_TRACE)
    _LAST_RESULTS.clear()
    _LAST_RESULTS.append(res)
    out = np.concatenate([r["out"].reshape(B_CORE) for r in res.results])
    return out.reshape(BATCH, 1).astype(np.float32)
